# revision 30
# baseline (speedup 1.0000x reference)
"""Trainium2 Bass kernel for nn_Attention_75402445849133.

Dense per-batch attention:
  q = Wq @ x[b] + bq ; k = Wk @ x[b] + bk ; v = x[b] (unprojected)
  per head h (16 heads, d=64, S=128):
    scores = (q_h^T k_h) / 8 ; attn = softmax(scores) ; out_h = attn @ v_h^T
  score[b, f] = sum_s out[f, s] * Wo[s] + bo

Sharded data-parallel over batch B=256 across 8 NeuronCores (32 b/core).
All matmul operands fp16 (fp32 PSUM accumulation).

Key tricks:
  - scores computed TRANSPOSED (t on partitions) so softmax denominator and
    the AV matmul both contract over t on partitions with no attn transpose.
  - x[b]^T passed from the host with a constant ones column appended after
    each head's 64 columns, so the AV matmul (stationary = exp(scores)) also
    emits the softmax denominator column from the same stationary load.
  - no max-subtraction in softmax (scores are O(1) by construction).
  - softmax division via DVE reciprocal + broadcast multiply (per-partition).
  - final f-projection: Wo vector stationary over the normalized head
    outputs; TWO batches per PSUM tile using 4 PE column groups (0/32/64/96)
    so all 4 512-col matmuls run concurrently on the PE.
  - PSUM bank discipline: separate pools for QK accumulation (3 banks),
    scores+finals (3), uout quads (2); matmuls into one bank share a PE row
    group; even/odd head parities use separate banks and run concurrently.
  - fine-grained software pipelining: attention work (scores quads, exp,
    uout quads, finals) is chopped into small units and interleaved between
    the QK psum-groups of the NEXT group, so PSUM drains (ACT exp, DVE
    normalize) never gate the PE. Scores for the first half of the heads
    (oc=0, mt 0-3) are emitted mid-QK of their own group to shorten the
    pipeline tail.
"""

import sys
import types
from collections import deque

import numpy as np

from concourse import bass, bacc, bass_isa, tile, mybir
from concourse.bass_utils import run_bass_kernel_spmd


def _ensure_axon_hooks():
    """Provide antenv.axon_hooks if the image lacks it (needed for trace=True)."""
    try:
        import antenv.axon_hooks  # noqa: F401

        return
    except ImportError:
        pass
    import antenv

    mod = types.ModuleType("antenv.axon_hooks")
    mod._hook = None
    mod.set_axon_ntff_profile_hook = lambda h: setattr(mod, "_hook", h)
    mod.get_axon_ntff_profile_hook = lambda: mod._hook
    sys.modules["antenv.axon_hooks"] = mod
    antenv.axon_hooks = mod
    try:
        from trn_agent_boot.trn_boot import _ntff_profile_via_ctypes

        hook = _ntff_profile_via_ctypes("/opt/axon/libaxon_pjrt.so")
        if hook is not None:
            mod._hook = hook
    except Exception:
        pass


_ensure_axon_hooks()

F16 = mybir.dt.float16
F32 = mybir.dt.float32

N_CORES = 8
B = 256
F_IN = 1024
HID = 1024
H = 16
S = 128
D = 64  # head dim (both q/k and v)
KT = 8  # k tiles (F_IN / 128)
MT = 8  # m tiles (HID / 128)
TEMP = 8.0

TRACE = False  # test.py sets this for profiling runs


def build_bass(n_groups=8, G=4):
    """Build the per-core Bass graph. NB = n_groups * G local batches."""
    NB = n_groups * G
    NQK = G * S  # moving free dim of the QK matmuls

    nc = bacc.Bacc(None, target_bir_lowering=False)

    # host-prepared inputs (per core)
    xr = nc.dram_tensor("xr", [n_groups, 128, KT, G, S], F16, kind="ExternalInput")
    # x[b]^T per batch with a ones column after each head's 64 cols (baked on host)
    xtr = nc.dram_tensor("xtr", [n_groups, 128, G, H, D + 1], F16, kind="ExternalInput")
    wqt = nc.dram_tensor("wqt", [MT, 128, KT, 128], F16, kind="ExternalInput")
    wkt = nc.dram_tensor("wkt", [MT, 128, KT, 128], F16, kind="ExternalInput")
    bqr = nc.dram_tensor("bqr", [128, MT], F32, kind="ExternalInput")
    bkr = nc.dram_tensor("bkr", [128, MT], F32, kind="ExternalInput")
    wo16 = nc.dram_tensor("wo16", [128, 1], F16, kind="ExternalInput")
    out = nc.dram_tensor("out", [NB, F_IN], F32, kind="ExternalOutput")

    with tile.TileContext(nc) as tc:
        with (
            tc.tile_pool(name="consts", bufs=1) as cpool,
            tc.tile_pool(name="xp", bufs=2) as xpool,
            tc.tile_pool(name="xtp", bufs=3) as xtpool,
            tc.tile_pool(name="qkp", bufs=4) as qkpool,
            tc.tile_pool(name="ep", bufs=24) as epool,
            tc.tile_pool(name="wfp", bufs=4) as wfpool,
            tc.tile_pool(name="uop", bufs=8) as uopool,
            tc.tile_pool(name="orow", bufs=2) as orowpool,
            tc.tile_pool(name="ps_qk", bufs=3, space="PSUM") as ps_qk,
            tc.tile_pool(name="ps_sc", bufs=3, space="PSUM") as ps_sc,
            tc.tile_pool(name="ps_uo", bufs=2, space="PSUM") as ps_uo,
        ):
            # ---- persistent tiles ----
            wq_ts = [
                cpool.tile([128, KT, 128], F16, name=f"wq{mt}", tag=f"wq{mt}")
                for mt in range(MT)
            ]
            wk_ts = [
                cpool.tile([128, KT, 128], F16, name=f"wk{mt}", tag=f"wk{mt}")
                for mt in range(MT)
            ]
            bq_t = cpool.tile([128, MT], F32, tag="bq")
            bk_t = cpool.tile([128, MT], F32, tag="bk")
            wo_t = cpool.tile([128, 1], F16, tag="wo")
            zero_t = cpool.tile([128, 1], F32, tag="zero")

            nc.vector.memset(zero_t[:], 0.0)

            # Head DMAs: each dma_start costs ~0.6-1.3us of serial setup on
            # its engine's HWDGE queue, so spread across three queues and
            # put the small tensors the pipeline blocks on (biases) first.
            # sync: x group 0 + biases; vector: weights; scalar: xT + wo.
            x16_first = xpool.tile([128, KT, G, S], F16, tag="x16")
            nc.sync.dma_start(
                x16_first[:, 0:2],
                xr[0, :, 0:2].rearrange("p kt g s -> p (kt g s)"),
            )
            nc.sync.dma_start(
                x16_first[:, 2:5],
                xr[0, :, 2:5].rearrange("p kt g s -> p (kt g s)"),
            )
            nc.sync.dma_start(
                x16_first[:, 5:],
                xr[0, :, 5:].rearrange("p kt g s -> p (kt g s)"),
            )
            nc.sync.dma_start(bq_t[:], bqr[:])
            nc.sync.dma_start(bk_t[:], bkr[:])
            # only wq0 before the blocker: the first matmul needs it, but
            # wk0 isn't consumed until slot 1 (~2us later) — keep the early
            # DMA bandwidth for the x16 stream
            nc.scalar.dma_start(
                wq_ts[0][:], wqt[0].rearrange("p kt m -> p (kt m)")
            )
            # blocker: holds the remaining weight-bulk DMA setups until the
            # first x group has landed, so x16(0) gets full DMA bandwidth
            dummy_t = cpool.tile([1, 1], F16, tag="dummy")
            nc.scalar.activation(
                dummy_t[:],
                x16_first[0:1, KT - 1, G - 1, S - 1 : S],
                mybir.ActivationFunctionType.Copy,
                bias=0.0,
                scale=1.0,
            )
            nc.scalar.dma_start(
                wk_ts[0][:], wkt[0].rearrange("p kt m -> p (kt m)")
            )
            for mt in range(1, MT):
                nc.scalar.dma_start(
                    wq_ts[mt][:], wqt[mt].rearrange("p kt m -> p (kt m)")
                )
                nc.scalar.dma_start(
                    wk_ts[mt][:], wkt[mt].rearrange("p kt m -> p (kt m)")
                )
            nc.gpsimd.dma_start(wo_t[:], wo16[:])
            xT_first = xtpool.tile([128, G, H, D + 1], F16, tag="xT")
            nc.scalar.dma_start(
                xT_first[:], xtr[0].rearrange("p g h d -> p (g h d)")
            )

            # ---- pipelined attention work units ----
            # Two queues so each slot pops uout/finals before scores; units
            # are clumped on every other (mt, proj) slot so their stationary
            # loads hide under the adjacent 512-col QK matmuls.
            uo_q = deque()
            sc_q = deque()
            equads = {}  # (b_loc, oc, par) -> E tile
            uo_map = {}  # b_loc -> uo_sc tile

            def pop_slot(n_uo, n_sc):
                for _ in range(n_uo):
                    if uo_q:
                        uo_q.popleft()()
                for _ in range(n_sc):
                    if sc_q:
                        sc_q.popleft()()

            def mk_scores(b_loc, g, oc, q_sb, k_sb):
                # one oc-quad of one batch: 8 matmuls (4 j x 2 par) + 2 exps.
                # Even heads -> PE rows 0-63 (bank A), odd -> 64-127 (bank B):
                # the two parities run concurrently on the PE.
                def f():
                    ps_e = ps_sc.tile([128, 4 * S], F32, name=f"pse{b_loc}_{oc}", tag="sc")
                    ps_o = ps_sc.tile([128, 4 * S], F32, name=f"pso{b_loc}_{oc}", tag="sc")
                    for j in range(4):
                        mt = oc * 4 + j
                        for par, ps_s in ((0, ps_e), (1, ps_o)):
                            po = par * D
                            # scoresT[t, s] = sum_d k[d,t] * q[d,s]
                            nc.tensor.matmul(
                                ps_s[:, j * S : (j + 1) * S],
                                k_sb[po : po + D, mt, g * S : (g + 1) * S],
                                q_sb[po : po + D, mt, g * S : (g + 1) * S],
                            )
                    for par, ps_s in ((0, ps_e), (1, ps_o)):
                        E = epool.tile([128, 4 * S], F16, name=f"E{b_loc}_{oc}_{par}", tag="E")
                        nc.scalar.activation(
                            E[:],
                            ps_s[:],
                            mybir.ActivationFunctionType.Exp,
                            bias=zero_t[:, 0:1],
                            scale=1.0 / TEMP,
                        )
                        equads[(b_loc, oc, par)] = E

                return f

            def mk_uout(b_loc, g, oc, par, xT4, pool=None):
                # one parity-quad: 4 AV matmuls (65 cols: 64 d + ones col
                # giving the softmax denominator), then DVE normalize into
                # the batch's uo_sc tile. `pool` overrides the PSUM pool:
                # the tail drain borrows ps_qk's then-idle banks so
                # back-to-back quads don't gate on the DVE normalize chain.
                def f():
                    E = equads.pop((b_loc, oc, par))
                    ps_u = (pool or ps_uo).tile(
                        [128, 4, D + 1],
                        F32,
                        name=f"psu{b_loc}_{oc}_{par}",
                        tag="qk" if pool is not None else "uo",
                    )
                    heads = [2 * (oc * 4 + j) + par for j in range(4)]
                    for hi, h in enumerate(heads):
                        nc.tensor.matmul(
                            ps_u[:, hi, :],
                            E[:, hi * S : (hi + 1) * S],
                            xT4[:, g, h, :],
                        )
                    rc = wfpool.tile([128, 4], F32, name=f"rc{b_loc}_{oc}_{par}", tag="rc")
                    nc.vector.reciprocal(rc[:], ps_u[:, :, D])
                    if b_loc not in uo_map:
                        uo_map[b_loc] = uopool.tile([128, H * D], F16, name=f"uo{b_loc}", tag="uosc")
                    uo_view = uo_map[b_loc][:].rearrange(
                        "p (pair par d) -> p pair par d", par=2, d=D
                    )
                    nc.vector.tensor_mul(
                        uo_view[:, oc * 4 : (oc + 1) * 4, par, :],
                        ps_u[:, :, 0:D],
                        rc[:].unsqueeze(2).broadcast_to((128, 4, D)),
                    )

                return f

            def mk_finals(b0):
                # final projection for a PAIR of batches: 4 matmuls with the
                # Wo vector stationary at PE column groups 0/32/64/96 (all
                # concurrent), one ACT copy, 4 output DMAs.
                def f():
                    uo0 = uo_map.pop(b0)
                    uo1 = uo_map.pop(b0 + 1)
                    ps_f = ps_sc.tile([128, 512], F32, name=f"psf{b0}", tag="sc")
                    for ci, src in enumerate(
                        (uo0[:, 0:512], uo0[:, 512:1024], uo1[:, 0:512], uo1[:, 512:1024])
                    ):
                        nc.tensor.matmul(
                            ps_f[32 * ci : 32 * ci + 1, :],
                            wo_t[:],
                            src,
                            tile_position=(0, 32 * ci),
                        )
                    orow = orowpool.tile([128, 512], F32, name=f"orow{b0}", tag="orow")
                    nc.scalar.activation(
                        orow[0:97, :],
                        ps_f[0:97, :],
                        mybir.ActivationFunctionType.Copy,
                        bias=0.0,
                        scale=1.0,
                    )
                    # one strided DMA: orow partitions (0,32,64,96) -> the
                    # two batches' 2x512 output halves
                    nc.sync.dma_start(
                        out[b0 : b0 + 2, :].rearrange("b (h f) -> (b h) f", h=2),
                        orow[:].rearrange("(a c) f -> a c f", c=32)[:, 0, :],
                    )

                return f

            # pops per (mt, proj) slot of a group's QK emission; uout+finals
            # sum to 18, scores to 8 = units pushed per steady-state group.
            UO_SCHED = [0, 2, 1, 2, 1, 2, 1, 1, 0, 2, 1, 1, 1, 1, 1, 1]
            SC_SCHED = [0, 1, 0, 1, 0, 1, 0, 1, 0, 1, 0, 1, 0, 1, 0, 1]

            prev = None  # (q_sb, k_sb, xT4) of previous group

            for grp in range(n_groups):
                if grp == 0:
                    x16, xT4 = x16_first, xT_first
                else:
                    x16 = xpool.tile([128, KT, G, S], F16, tag="x16")
                    nc.sync.dma_start(
                        x16[:], xr[grp].rearrange("p kt g s -> p (kt g s)")
                    )
                    xT4 = xtpool.tile([128, G, H, D + 1], F16, tag="xT")
                    nc.sync.dma_start(
                        xT4[:], xtr[grp].rearrange("p g h d -> p (g h d)")
                    )

                # ---- QK projections: q/k = W @ x (+bias), fp16 out ----
                q_sb = qkpool.tile([128, MT, NQK], F16, tag="q")
                k_sb = qkpool.tile([128, MT, NQK], F16, tag="k")
                slot = 0
                for mt in range(MT):
                    for w_ts, b_t, dst in (
                        (wq_ts, bq_t, q_sb),
                        (wk_ts, bk_t, k_sb),
                    ):
                        ps = ps_qk.tile([128, NQK], F32, tag="qk")
                        for kt in range(KT):
                            nc.tensor.matmul(
                                ps[:],
                                w_ts[mt][:, kt, :],
                                x16[:, kt, :, :].rearrange("p g s -> p (g s)"),
                                start=(kt == 0),
                                stop=(kt == KT - 1),
                            )
                        # bias add + fp16 cast (DVE)
                        nc.vector.tensor_scalar_add(
                            dst[:, mt, :], ps[:], b_t[:, mt : mt + 1]
                        )
                        pop_slot(UO_SCHED[slot], SC_SCHED[slot])
                        slot += 1
                    if mt == 3:
                        # first-half heads' scores can start as soon as the
                        # mt 0-3 bias-adds land — shortens the pipeline tail
                        for g in range(G):
                            sc_q.append(mk_scores(grp * G + g, g, 0, q_sb, k_sb))
                        # prev group's odd-half uout: exps from its oc1
                        # scores (popped in slots 0-3) are done by now
                        if prev is not None:
                            for g in range(G):
                                pb = (grp - 1) * G + g
                                for par in (0, 1):
                                    uo_q.append(mk_uout(pb, g, 1, par, prev[2]))

                # ---- push the rest of this group's + prior group's work ----
                # interleaved so ps_sc allocations are spaced apart
                for g in range(G):
                    sc_q.append(mk_scores(grp * G + g, g, 1, q_sb, k_sb))
                    cb = grp * G + g
                    for par in (0, 1):
                        uo_q.append(mk_uout(cb, g, 0, par, xT4))
                    if prev is not None and g % 2 == 1:
                        uo_q.append(mk_finals((grp - 1) * G + g - 1))
                prev = (q_sb, k_sb, xT4)

            # ---- tail: scores first (their exps are the tail critical
            # path); drain-time uout quads alternate PSUM pools (ps_uo +
            # the now-idle ps_qk) so the DVE normalize chain never gates.
            while sc_q:
                sc_q.popleft()()
            drain_uo = list(uo_q)
            uo_q.clear()
            for g in range(G):
                pb = (n_groups - 1) * G + g
                for qi, par in enumerate((0, 1)):
                    drain_uo.append(
                        mk_uout(pb, g, 1, par, prev[2], ps_qk if qi else None)
                    )
                if g % 2 == 1:
                    drain_uo.append(mk_finals(pb - 1))
            for u in drain_uo:
                u()

    nc.compile()
    return nc


def prep_inputs(x, Wq, bq, Wk, bk, Wo, n_groups=8, G=4, n_cores=N_CORES):
    """Host-side shard + layout prep. Returns in_maps for run_bass_kernel_spmd."""
    x = np.asarray(x, dtype=np.float32)
    nb = n_groups * G
    x16 = x.astype(np.float16)
    # (c, grp, g, kt, p, s) -> (c, grp, p, kt, g, s)
    xr = (
        x16.reshape(n_cores, n_groups, G, KT, 128, S)
        .transpose(0, 1, 4, 3, 2, 5)
        .copy()
    )
    # x^T per batch with ones col per head: (c, grp, t, g, h, 65)
    xtr = np.ones((n_cores, n_groups, S, G, H, D + 1), dtype=np.float16)
    xtr[..., 0:D] = x16.reshape(n_cores, n_groups, G, H, D, S).transpose(
        0, 1, 5, 2, 3, 4
    )
    # W.T is (k, m); lay out as (mt, p, kt, 128) so each mt tile is one DMA
    wqt = np.ascontiguousarray(
        np.asarray(Wq, dtype=np.float32).T.reshape(KT, 128, MT, 128).transpose(2, 1, 0, 3)
    ).astype(np.float16)
    wkt = np.ascontiguousarray(
        np.asarray(Wk, dtype=np.float32).T.reshape(KT, 128, MT, 128).transpose(2, 1, 0, 3)
    ).astype(np.float16)
    bqr = np.ascontiguousarray(np.asarray(bq, dtype=np.float32).reshape(MT, 128).T)
    bkr = np.ascontiguousarray(np.asarray(bk, dtype=np.float32).reshape(MT, 128).T)
    wo_a = np.asarray(Wo, dtype=np.float32).reshape(128, 1).astype(np.float16)
    in_maps = []
    for c in range(n_cores):
        in_maps.append(
            {
                "xr": xr[c],
                "xtr": xtr[c],
                "wqt": wqt,
                "wkt": wkt,
                "bqr": bqr,
                "bkr": bkr,
                "wo16": wo_a,
            }
        )
    return in_maps


_NC_CACHE = {}


def kernel(x, Wq, bq, Wk, bk, Wo, bo):
    key = "full"
    if key not in _NC_CACHE:
        _NC_CACHE[key] = build_bass()
    nc = _NC_CACHE[key]

    in_maps = prep_inputs(x, Wq, bq, Wk, bk, Wo)
    res = run_bass_kernel_spmd(nc, in_maps, core_ids=list(range(N_CORES)), trace=TRACE)
    kernel.last_result = res
    out = np.concatenate([res.results[c]["out"] for c in range(N_CORES)], axis=0)
    out = out + np.float32(np.asarray(bo).reshape(-1)[0])
    return out.astype(np.float32)


# revision 31
# speedup vs baseline: 1.0015x; 1.0015x over previous
"""Trainium2 Bass kernel for nn_Attention_75402445849133.

Dense per-batch attention:
  q = Wq @ x[b] + bq ; k = Wk @ x[b] + bk ; v = x[b] (unprojected)
  per head h (16 heads, d=64, S=128):
    scores = (q_h^T k_h) / 8 ; attn = softmax(scores) ; out_h = attn @ v_h^T
  score[b, f] = sum_s out[f, s] * Wo[s] + bo

Sharded data-parallel over batch B=256 across 8 NeuronCores (32 b/core).
All matmul operands fp16 (fp32 PSUM accumulation).

Key tricks:
  - scores computed TRANSPOSED (t on partitions) so softmax denominator and
    the AV matmul both contract over t on partitions with no attn transpose.
  - x[b]^T passed from the host with a constant ones column appended after
    each head's 64 columns, so the AV matmul (stationary = exp(scores)) also
    emits the softmax denominator column from the same stationary load.
  - no max-subtraction in softmax (scores are O(1) by construction).
  - softmax division via DVE reciprocal + broadcast multiply (per-partition).
  - final f-projection: Wo vector stationary over the normalized head
    outputs; TWO batches per PSUM tile using 4 PE column groups (0/32/64/96)
    so all 4 512-col matmuls run concurrently on the PE.
  - PSUM bank discipline: separate pools for QK accumulation (3 banks),
    scores+finals (3), uout quads (2); matmuls into one bank share a PE row
    group; even/odd head parities use separate banks and run concurrently.
  - fine-grained software pipelining: attention work (scores quads, exp,
    uout quads, finals) is chopped into small units and interleaved between
    the QK psum-groups of the NEXT group, so PSUM drains (ACT exp, DVE
    normalize) never gate the PE. Scores for the first half of the heads
    (oc=0, mt 0-3) are emitted mid-QK of their own group to shorten the
    pipeline tail.
"""

import sys
import types
from collections import deque

import numpy as np

from concourse import bass, bacc, bass_isa, tile, mybir
from concourse.bass_utils import run_bass_kernel_spmd


def _ensure_axon_hooks():
    """Provide antenv.axon_hooks if the image lacks it (needed for trace=True)."""
    try:
        import antenv.axon_hooks  # noqa: F401

        return
    except ImportError:
        pass
    import antenv

    mod = types.ModuleType("antenv.axon_hooks")
    mod._hook = None
    mod.set_axon_ntff_profile_hook = lambda h: setattr(mod, "_hook", h)
    mod.get_axon_ntff_profile_hook = lambda: mod._hook
    sys.modules["antenv.axon_hooks"] = mod
    antenv.axon_hooks = mod
    try:
        from trn_agent_boot.trn_boot import _ntff_profile_via_ctypes

        hook = _ntff_profile_via_ctypes("/opt/axon/libaxon_pjrt.so")
        if hook is not None:
            mod._hook = hook
    except Exception:
        pass


_ensure_axon_hooks()

F16 = mybir.dt.float16
F32 = mybir.dt.float32

N_CORES = 8
B = 256
F_IN = 1024
HID = 1024
H = 16
S = 128
D = 64  # head dim (both q/k and v)
KT = 8  # k tiles (F_IN / 128)
MT = 8  # m tiles (HID / 128)
TEMP = 8.0

TRACE = False  # test.py sets this for profiling runs


def build_bass(n_groups=8, G=4):
    """Build the per-core Bass graph. NB = n_groups * G local batches."""
    NB = n_groups * G
    NQK = G * S  # moving free dim of the QK matmuls

    nc = bacc.Bacc(None, target_bir_lowering=False)

    # host-prepared inputs (per core)
    xr = nc.dram_tensor("xr", [n_groups, 128, KT, G, S], F16, kind="ExternalInput")
    # x[b]^T per batch with a ones column after each head's 64 cols (baked on host)
    xtr = nc.dram_tensor("xtr", [n_groups, 128, G, H, D + 1], F16, kind="ExternalInput")
    wqt = nc.dram_tensor("wqt", [MT, 128, KT, 128], F16, kind="ExternalInput")
    wkt = nc.dram_tensor("wkt", [MT, 128, KT, 128], F16, kind="ExternalInput")
    bqr = nc.dram_tensor("bqr", [128, MT], F32, kind="ExternalInput")
    bkr = nc.dram_tensor("bkr", [128, MT], F32, kind="ExternalInput")
    wo16 = nc.dram_tensor("wo16", [128, 1], F16, kind="ExternalInput")
    out = nc.dram_tensor("out", [NB, F_IN], F32, kind="ExternalOutput")

    with tile.TileContext(nc) as tc:
        with (
            tc.tile_pool(name="consts", bufs=1) as cpool,
            tc.tile_pool(name="xp", bufs=2) as xpool,
            tc.tile_pool(name="xtp", bufs=3) as xtpool,
            tc.tile_pool(name="qkp", bufs=4) as qkpool,
            tc.tile_pool(name="ep", bufs=24) as epool,
            tc.tile_pool(name="wfp", bufs=4) as wfpool,
            tc.tile_pool(name="uop", bufs=8) as uopool,
            tc.tile_pool(name="orow", bufs=2) as orowpool,
            tc.tile_pool(name="ps_qk", bufs=3, space="PSUM") as ps_qk,
            tc.tile_pool(name="ps_sc", bufs=3, space="PSUM") as ps_sc,
            tc.tile_pool(name="ps_uo", bufs=2, space="PSUM") as ps_uo,
        ):
            # ---- persistent tiles ----
            wq_ts = [
                cpool.tile([128, KT, 128], F16, name=f"wq{mt}", tag=f"wq{mt}")
                for mt in range(MT)
            ]
            wk_ts = [
                cpool.tile([128, KT, 128], F16, name=f"wk{mt}", tag=f"wk{mt}")
                for mt in range(MT)
            ]
            bq_t = cpool.tile([128, MT], F32, tag="bq")
            bk_t = cpool.tile([128, MT], F32, tag="bk")
            wo_t = cpool.tile([128, 1], F16, tag="wo")
            zero_t = cpool.tile([128, 1], F32, tag="zero")

            nc.vector.memset(zero_t[:], 0.0)

            # Head DMAs: each dma_start costs ~0.6-1.3us of serial setup on
            # its engine's HWDGE queue, so spread across three queues and
            # put the small tensors the pipeline blocks on (biases) first.
            # sync: x group 0 + biases; vector: weights; scalar: xT + wo.
            x16_first = xpool.tile([128, KT, G, S], F16, tag="x16")
            nc.sync.dma_start(
                x16_first[:, 0:2],
                xr[0, :, 0:2].rearrange("p kt g s -> p (kt g s)"),
            )
            nc.sync.dma_start(
                x16_first[:, 2:5],
                xr[0, :, 2:5].rearrange("p kt g s -> p (kt g s)"),
            )
            nc.sync.dma_start(
                x16_first[:, 5:],
                xr[0, :, 5:].rearrange("p kt g s -> p (kt g s)"),
            )
            nc.sync.dma_start(bq_t[:], bqr[:])
            nc.sync.dma_start(bk_t[:], bkr[:])
            # only wq0 before the blocker: the first matmul needs it, but
            # wk0 isn't consumed until slot 1 (~2us later) — keep the early
            # DMA bandwidth for the x16 stream
            nc.scalar.dma_start(
                wq_ts[0][:], wqt[0].rearrange("p kt m -> p (kt m)")
            )
            # blocker: holds the remaining weight-bulk DMA setups until the
            # first x group has landed, so x16(0) gets full DMA bandwidth
            dummy_t = cpool.tile([1, 1], F16, tag="dummy")
            nc.scalar.activation(
                dummy_t[:],
                x16_first[0:1, KT - 1, G - 1, S - 1 : S],
                mybir.ActivationFunctionType.Copy,
                bias=0.0,
                scale=1.0,
            )
            nc.scalar.dma_start(
                wk_ts[0][:], wkt[0].rearrange("p kt m -> p (kt m)")
            )
            for mt in range(1, MT):
                nc.scalar.dma_start(
                    wq_ts[mt][:], wqt[mt].rearrange("p kt m -> p (kt m)")
                )
                nc.scalar.dma_start(
                    wk_ts[mt][:], wkt[mt].rearrange("p kt m -> p (kt m)")
                )
            nc.gpsimd.dma_start(wo_t[:], wo16[:])
            xT_first = xtpool.tile([128, G, H, D + 1], F16, tag="xT")
            nc.scalar.dma_start(
                xT_first[:], xtr[0].rearrange("p g h d -> p (g h d)")
            )

            # ---- pipelined attention work units ----
            # Two queues so each slot pops uout/finals before scores; units
            # are clumped on every other (mt, proj) slot so their stationary
            # loads hide under the adjacent 512-col QK matmuls.
            uo_q = deque()
            sc_q = deque()
            equads = {}  # (b_loc, oc, par) -> E tile
            uo_map = {}  # b_loc -> uo_sc tile

            def pop_slot(n_uo, n_sc):
                for _ in range(n_uo):
                    if uo_q:
                        uo_q.popleft()()
                for _ in range(n_sc):
                    if sc_q:
                        sc_q.popleft()()

            def mk_scores(b_loc, g, oc, q_sb, k_sb):
                # one oc-quad of one batch: 8 matmuls (4 j x 2 par) + 2 exps.
                # Even heads -> PE rows 0-63 (bank A), odd -> 64-127 (bank B):
                # the two parities run concurrently on the PE.
                def f():
                    ps_e = ps_sc.tile([128, 4 * S], F32, name=f"pse{b_loc}_{oc}", tag="sc")
                    ps_o = ps_sc.tile([128, 4 * S], F32, name=f"pso{b_loc}_{oc}", tag="sc")
                    for j in range(4):
                        mt = oc * 4 + j
                        for par, ps_s in ((0, ps_e), (1, ps_o)):
                            po = par * D
                            # scoresT[t, s] = sum_d k[d,t] * q[d,s]
                            nc.tensor.matmul(
                                ps_s[:, j * S : (j + 1) * S],
                                k_sb[po : po + D, mt, g * S : (g + 1) * S],
                                q_sb[po : po + D, mt, g * S : (g + 1) * S],
                            )
                    for par, ps_s in ((0, ps_e), (1, ps_o)):
                        E = epool.tile([128, 4 * S], F16, name=f"E{b_loc}_{oc}_{par}", tag="E")
                        nc.scalar.activation(
                            E[:],
                            ps_s[:],
                            mybir.ActivationFunctionType.Exp,
                            bias=zero_t[:, 0:1],
                            scale=1.0 / TEMP,
                        )
                        equads[(b_loc, oc, par)] = E

                return f

            def mk_uout(b_loc, g, oc, par, xT4, pool=None):
                # one parity-quad: 4 AV matmuls (65 cols: 64 d + ones col
                # giving the softmax denominator), then DVE normalize into
                # the batch's uo_sc tile. `pool` overrides the PSUM pool:
                # the tail drain borrows ps_qk's then-idle banks so
                # back-to-back quads don't gate on the DVE normalize chain.
                def f():
                    E = equads.pop((b_loc, oc, par))
                    ps_u = (pool or ps_uo).tile(
                        [128, 4, D + 1],
                        F32,
                        name=f"psu{b_loc}_{oc}_{par}",
                        tag="qk" if pool is not None else "uo",
                    )
                    heads = [2 * (oc * 4 + j) + par for j in range(4)]
                    for hi, h in enumerate(heads):
                        nc.tensor.matmul(
                            ps_u[:, hi, :],
                            E[:, hi * S : (hi + 1) * S],
                            xT4[:, g, h, :],
                        )
                    rc = wfpool.tile([128, 4], F32, name=f"rc{b_loc}_{oc}_{par}", tag="rc")
                    nc.vector.reciprocal(rc[:], ps_u[:, :, D])
                    if b_loc not in uo_map:
                        uo_map[b_loc] = uopool.tile([128, H * D], F16, name=f"uo{b_loc}", tag="uosc")
                    uo_view = uo_map[b_loc][:].rearrange(
                        "p (pair par d) -> p pair par d", par=2, d=D
                    )
                    nc.vector.tensor_mul(
                        uo_view[:, oc * 4 : (oc + 1) * 4, par, :],
                        ps_u[:, :, 0:D],
                        rc[:].unsqueeze(2).broadcast_to((128, 4, D)),
                    )

                return f

            def mk_finals(b0):
                # final projection for a PAIR of batches: 4 matmuls with the
                # Wo vector stationary at PE column groups 0/32/64/96 (all
                # concurrent), one ACT copy, 4 output DMAs.
                def f():
                    uo0 = uo_map.pop(b0)
                    uo1 = uo_map.pop(b0 + 1)
                    ps_f = ps_sc.tile([128, 512], F32, name=f"psf{b0}", tag="sc")
                    for ci, src in enumerate(
                        (uo0[:, 0:512], uo0[:, 512:1024], uo1[:, 0:512], uo1[:, 512:1024])
                    ):
                        nc.tensor.matmul(
                            ps_f[32 * ci : 32 * ci + 1, :],
                            wo_t[:],
                            src,
                            tile_position=(0, 32 * ci),
                        )
                    orow = orowpool.tile([128, 512], F32, name=f"orow{b0}", tag="orow")
                    nc.scalar.activation(
                        orow[0:97, :],
                        ps_f[0:97, :],
                        mybir.ActivationFunctionType.Copy,
                        bias=0.0,
                        scale=1.0,
                    )
                    # one strided DMA: orow partitions (0,32,64,96) -> the
                    # two batches' 2x512 output halves
                    nc.sync.dma_start(
                        out[b0 : b0 + 2, :].rearrange("b (h f) -> (b h) f", h=2),
                        orow[:].rearrange("(a c) f -> a c f", c=32)[:, 0, :],
                    )

                return f

            # pops per (mt, proj) slot of a group's QK emission; uout+finals
            # sum to 18, scores to 8 = units pushed per steady-state group.
            UO_SCHED = [0, 2, 1, 2, 1, 2, 1, 1, 0, 2, 1, 1, 1, 1, 1, 1]
            SC_SCHED = [0, 1, 0, 1, 0, 1, 0, 1, 0, 1, 0, 1, 0, 1, 0, 1]

            prev = None  # (q_sb, k_sb, xT4) of previous group

            for grp in range(n_groups):
                if grp == 0:
                    x16, xT4 = x16_first, xT_first
                else:
                    x16 = xpool.tile([128, KT, G, S], F16, tag="x16")
                    nc.sync.dma_start(
                        x16[:], xr[grp].rearrange("p kt g s -> p (kt g s)")
                    )
                    xT4 = xtpool.tile([128, G, H, D + 1], F16, tag="xT")
                    nc.sync.dma_start(
                        xT4[:], xtr[grp].rearrange("p g h d -> p (g h d)")
                    )

                # ---- QK projections: q/k = W @ x (+bias), fp16 out ----
                q_sb = qkpool.tile([128, MT, NQK], F16, tag="q")
                k_sb = qkpool.tile([128, MT, NQK], F16, tag="k")
                slot = 0
                for mt in range(MT):
                    for w_ts, b_t, dst in (
                        (wk_ts, bk_t, k_sb),
                        (wq_ts, bq_t, q_sb),
                    ):
                        ps = ps_qk.tile([128, NQK], F32, tag="qk")
                        for kt in range(KT):
                            nc.tensor.matmul(
                                ps[:],
                                w_ts[mt][:, kt, :],
                                x16[:, kt, :, :].rearrange("p g s -> p (g s)"),
                                start=(kt == 0),
                                stop=(kt == KT - 1),
                            )
                        # bias add + fp16 cast (DVE)
                        nc.vector.tensor_scalar_add(
                            dst[:, mt, :], ps[:], b_t[:, mt : mt + 1]
                        )
                        pop_slot(UO_SCHED[slot], SC_SCHED[slot])
                        slot += 1
                    if mt == 3:
                        # first-half heads' scores can start as soon as the
                        # mt 0-3 bias-adds land — shortens the pipeline tail
                        for g in range(G):
                            sc_q.append(mk_scores(grp * G + g, g, 0, q_sb, k_sb))
                        # prev group's odd-half uout: exps from its oc1
                        # scores (popped in slots 0-3) are done by now
                        if prev is not None:
                            for g in range(G):
                                pb = (grp - 1) * G + g
                                for par in (0, 1):
                                    uo_q.append(mk_uout(pb, g, 1, par, prev[2]))

                # ---- push the rest of this group's + prior group's work ----
                # interleaved so ps_sc allocations are spaced apart
                for g in range(G):
                    sc_q.append(mk_scores(grp * G + g, g, 1, q_sb, k_sb))
                    cb = grp * G + g
                    for par in (0, 1):
                        uo_q.append(mk_uout(cb, g, 0, par, xT4))
                    if prev is not None and g % 2 == 1:
                        uo_q.append(mk_finals((grp - 1) * G + g - 1))
                prev = (q_sb, k_sb, xT4)

            # ---- tail: scores first (their exps are the tail critical
            # path); drain-time uout quads alternate PSUM pools (ps_uo +
            # the now-idle ps_qk) so the DVE normalize chain never gates.
            while sc_q:
                sc_q.popleft()()
            drain_uo = list(uo_q)
            uo_q.clear()
            for g in range(G):
                pb = (n_groups - 1) * G + g
                for qi, par in enumerate((0, 1)):
                    drain_uo.append(
                        mk_uout(pb, g, 1, par, prev[2], ps_qk if qi else None)
                    )
                if g % 2 == 1:
                    drain_uo.append(mk_finals(pb - 1))
            for u in drain_uo:
                u()

    nc.compile()
    return nc


def prep_inputs(x, Wq, bq, Wk, bk, Wo, n_groups=8, G=4, n_cores=N_CORES):
    """Host-side shard + layout prep. Returns in_maps for run_bass_kernel_spmd."""
    x = np.asarray(x, dtype=np.float32)
    nb = n_groups * G
    x16 = x.astype(np.float16)
    # (c, grp, g, kt, p, s) -> (c, grp, p, kt, g, s)
    xr = (
        x16.reshape(n_cores, n_groups, G, KT, 128, S)
        .transpose(0, 1, 4, 3, 2, 5)
        .copy()
    )
    # x^T per batch with ones col per head: (c, grp, t, g, h, 65)
    xtr = np.ones((n_cores, n_groups, S, G, H, D + 1), dtype=np.float16)
    xtr[..., 0:D] = x16.reshape(n_cores, n_groups, G, H, D, S).transpose(
        0, 1, 5, 2, 3, 4
    )
    # W.T is (k, m); lay out as (mt, p, kt, 128) so each mt tile is one DMA
    wqt = np.ascontiguousarray(
        np.asarray(Wq, dtype=np.float32).T.reshape(KT, 128, MT, 128).transpose(2, 1, 0, 3)
    ).astype(np.float16)
    wkt = np.ascontiguousarray(
        np.asarray(Wk, dtype=np.float32).T.reshape(KT, 128, MT, 128).transpose(2, 1, 0, 3)
    ).astype(np.float16)
    bqr = np.ascontiguousarray(np.asarray(bq, dtype=np.float32).reshape(MT, 128).T)
    bkr = np.ascontiguousarray(np.asarray(bk, dtype=np.float32).reshape(MT, 128).T)
    wo_a = np.asarray(Wo, dtype=np.float32).reshape(128, 1).astype(np.float16)
    in_maps = []
    for c in range(n_cores):
        in_maps.append(
            {
                "xr": xr[c],
                "xtr": xtr[c],
                "wqt": wqt,
                "wkt": wkt,
                "bqr": bqr,
                "bkr": bkr,
                "wo16": wo_a,
            }
        )
    return in_maps


_NC_CACHE = {}


def kernel(x, Wq, bq, Wk, bk, Wo, bo):
    key = "full"
    if key not in _NC_CACHE:
        _NC_CACHE[key] = build_bass()
    nc = _NC_CACHE[key]

    in_maps = prep_inputs(x, Wq, bq, Wk, bk, Wo)
    res = run_bass_kernel_spmd(nc, in_maps, core_ids=list(range(N_CORES)), trace=TRACE)
    kernel.last_result = res
    out = np.concatenate([res.results[c]["out"] for c in range(N_CORES)], axis=0)
    out = out + np.float32(np.asarray(bo).reshape(-1)[0])
    return out.astype(np.float32)
